# revision 4
# baseline (speedup 1.0000x reference)
"""DCNv2 deformable-conv alignment kernel for 8 Trainium2 NeuronCores.

Sharding: core i handles (b = i//2, row-half = i%2) of the B=4, H=128 input.
Each core computes its half-image rows end-to-end:
  conv1 (128->64, 3x3) + lrelu -> conv2 (66->216, 3x3; two extra channels
  carry the row/col position maps) -> offsets/mask -> bilinear sampling of a
  zero-padded fp16 nbr image via GPSIMD ap_gather (d=2 x-pairs, no validity
  masking needed) -> modulated DCN matmul (corner sum folded into 4
  accumulating matmuls per tap) -> bias + lrelu.
"""
import sys

for _p in ("/opt/trn_rl_repo", "/root/.axon_site/_ro/trn_rl_repo"):
    if _p not in sys.path:
        sys.path.insert(0, _p)

import numpy as np

NF, G, K = 64, 8, 3
KK = K * K
CG = NF // G
B, H, W = 4, 128, 128
N_CORES = 8
HALF = H // 2            # rows per core
CH = 512                 # positions per chunk (4 image rows)
RPC = CH // W            # rows per chunk = 4
NCHUNK = HALF * W // CH  # 16
PAD = 2
PW = W + 2 * PAD         # 132 padded width/height
NE2 = PW * PW            # 17424 padded positions
NI = 2 * CH              # gather indices per tap (pos*2 + ypair), d=2 x-pairs
FB = 512.0               # positivity bias so int-convert+fix == floor

_compiled = None


def _build_program():
    import concourse.bacc as bacc
    import concourse.mybir as mybir
    import concourse.tile as tile
    from concourse.tile_rust import add_dep_helper

    dt = mybir.dt
    Alu = mybir.AluOpType
    Act = mybir.ActivationFunctionType

    nc = bacc.Bacc("TRN2", target_bir_lowering=False, debug=False,
                   num_devices=N_CORES)

    # ---- DRAM I/O ----
    conv_in_d = nc.dram_tensor("conv_in", [128, 68 * 130], dt.float16, kind="ExternalInput").ap()
    nbr_g_d = nc.dram_tensor("nbr_g", [128, 2 * NE2], dt.float16, kind="ExternalInput").ap()
    w1_d = nc.dram_tensor("w1", [128, KK * 64], dt.float16, kind="ExternalInput").ap()
    w2_d = nc.dram_tensor("w2", [66, 3 * KK * 72], dt.float16, kind="ExternalInput").ap()
    w3_d = nc.dram_tensor("w3", [128, KK * 64], dt.float16, kind="ExternalInput").ap()
    wrep_d = nc.dram_tensor("wrep", [72, KK * 128], dt.float16, kind="ExternalInput").ap()
    rwmap_d = nc.dram_tensor("rwmap", [2, 66 * 130], dt.float16, kind="ExternalInput").ap()
    by_d = nc.dram_tensor("by", [72, 1], dt.float32, kind="ExternalInput").ap()
    bx_d = nc.dram_tensor("bx", [72, 1], dt.float32, kind="ExternalInput").ap()
    bm_d = nc.dram_tensor("bm", [72, 1], dt.float32, kind="ExternalInput").ap()
    b1_d = nc.dram_tensor("b1", [64, 1], dt.float32, kind="ExternalInput").ap()
    b3_d = nc.dram_tensor("b3", [64, 1], dt.float32, kind="ExternalInput").ap()
    e0_d = nc.dram_tensor("e0", [64, 1], dt.float32, kind="ExternalInput").ap()
    e65_d = nc.dram_tensor("e65", [64, 1], dt.float32, kind="ExternalInput").ap()
    out_d = nc.dram_tensor("out", [64, HALF * W], dt.float32, kind="ExternalOutput").ap()

    f32 = dt.float32
    f16 = dt.float16

    def lrelu_stt(out_ap, in_ap):
        # out = max(0.1*x, x)
        nc.vector.scalar_tensor_tensor(out_ap, in_ap, 0.1, in_ap, Alu.mult, Alu.max)

    # Static SBUF for DMA-written / gather tiles: HWDGE-queue accesses are
    # under-synchronized vs Tile pool slot reuse, so fixed addresses + manual
    # deps.
    idxw = nc.alloc_sbuf_tensor("idxw_s", [128, KK * (NI // 16)], dt.int16).ap()
    st2 = nc.alloc_sbuf_tensor("st2_s", [72, 2 * 8 * 64], dt.int16).ap()
    g_out = nc.alloc_sbuf_tensor("g_out_s", [128, NI * 2], f16).ap()

    with tile.TileContext(nc) as tc:
        with tc.tile_pool(name="const", bufs=1) as cpool, \
             tc.tile_pool(name="psum", bufs=1, space="PSUM") as ppool:

            # ---- persistent loads ----
            nbr_sb = cpool.tile([128, 2 * NE2], f16)
            nc.sync.dma_start(nbr_sb[:], nbr_g_d[:])
            w1_sb = cpool.tile([128, KK * 64], f16)
            nc.sync.dma_start(w1_sb[:], w1_d[:])
            w2_sb = cpool.tile([66, 3 * KK * 72], f16)
            nc.sync.dma_start(w2_sb[:], w2_d[:])
            w3_sb = cpool.tile([128, KK * 64], f16)
            nc.sync.dma_start(w3_sb[:], w3_d[:])
            wrep_sb = cpool.tile([72, KK * 128], f16)
            nc.sync.dma_start(wrep_sb[:], wrep_d[:])
            by_sb = cpool.tile([72, 1], f32)
            nc.sync.dma_start(by_sb[:], by_d[:])
            bx_sb = cpool.tile([72, 1], f32)
            nc.sync.dma_start(bx_sb[:], bx_d[:])
            bm_sb = cpool.tile([72, 1], f32)
            nc.sync.dma_start(bm_sb[:], bm_d[:])
            b1_sb = cpool.tile([64, 1], f32)
            nc.sync.dma_start(b1_sb[:], b1_d[:])
            b3_sb = cpool.tile([64, 1], f32)
            nc.sync.dma_start(b3_sb[:], b3_d[:])
            e0_sb = cpool.tile([64, 1], f32)
            nc.sync.dma_start(e0_sb[:], e0_d[:])
            e65_sb = cpool.tile([64, 1], f32)
            nc.sync.dma_start(e65_sb[:], e65_d[:])

            # ---- conv1: off_feat rows [-1, HALF+1) padded cols (130 wide);
            # partitions 64,65 hold the row/col position maps for conv2.
            off_sb = cpool.tile([66, 66 * 130], f16)
            nc.vector.memset(off_sb[:], 0.0)
            nc.sync.dma_start(off_sb[64:66, :], rwmap_d[:])
            off_v = off_sb[:].rearrange("p (r c) -> p r c", c=130)
            with tc.tile_pool(name="cin", bufs=1) as cinpool:
                conv_in_sb = cinpool.tile([128, 68 * 130], f16)
                nc.sync.dma_start(conv_in_sb[:], conv_in_d[:])
                cin_v = conv_in_sb[:].rearrange("p (r c) -> p r c", c=130)
                j0 = 0
                while j0 < 66:
                    nrow = min(4, 66 - j0)
                    ps1 = ppool.tile([64, nrow, 128], f32, tag="ps2_0")
                    for kt in range(KK):
                        ky, kx = kt // 3, kt % 3
                        rhs = cin_v[:, j0 + ky: j0 + ky + nrow, kx: kx + 128]
                        nc.tensor.matmul(ps1[:], w1_sb[:, kt * 64:(kt + 1) * 64],
                                         rhs, start=(kt == 0), stop=(kt == KK - 1))
                    scf = cinpool.tile([64, nrow, 128], f32, tag="scf")
                    nc.vector.tensor_scalar(scf[:], ps1[:], b1_sb[:, 0:1], None, Alu.add)
                    lrelu_stt(off_v[0:64, j0: j0 + nrow, 1:129], scf[:])
                    j0 += nrow
            # off_feat rows outside the image must be ZERO for conv2's
            # zero-padding semantics (row j=0 is global s-1; j=65 is s+65).
            nc.vector.tensor_scalar(off_sb[0:64, 0:130], off_sb[0:64, 0:130],
                                    e0_sb[:, 0:1], None, Alu.mult)
            nc.vector.tensor_scalar(off_sb[0:64, 65 * 130:66 * 130],
                                    off_sb[0:64, 65 * 130:66 * 130],
                                    e65_sb[:, 0:1], None, Alu.mult)

            # ---- per-chunk pipeline ----
            prev_gathers = [[]]   # gathers of previous chunk (WAR for idx DMAs)
            prev_dmas = [[]]      # idx DMAs of previous chunk (WAR for st2 casts)
            with tc.tile_pool(name="work", bufs=1) as wpool:
                for c in range(NCHUNK):
                    # conv2 -> three field psums [72, CH]; contraction 66
                    # (channels 64,65 add the row/col maps via the center tap)
                    ps_f = []
                    for f in range(3):
                        psf = ppool.tile([72, RPC, 128], f32, tag=f"ps2_{f}")
                        for kt in range(KK):
                            ky, kx = kt // 3, kt % 3
                            rhs = off_v[:, c * RPC + ky: c * RPC + ky + RPC, kx: kx + 128]
                            nc.tensor.matmul(
                                psf[:],
                                w2_sb[:, (f * KK + kt) * 72:(f * KK + kt + 1) * 72],
                                rhs, start=(kt == 0), stop=(kt == KK - 1))
                        ps_f.append(psf)

                    # qyb = row + dy + off_y + 512  (all biases folded into by)
                    qyb = wpool.tile([72, CH], f32, tag="qyb")
                    nc.vector.tensor_scalar(qyb[:], ps_f[0][:].rearrange("p a b -> p (a b)"),
                                            by_sb[:, 0:1], None, Alu.add)
                    qxb = wpool.tile([72, CH], f32, tag="qxb")
                    nc.vector.tensor_scalar(qxb[:], ps_f[1][:].rearrange("p a b -> p (a b)"),
                                            bx_sb[:, 0:1], None, Alu.add)
                    msk16 = wpool.tile([72, CH], f16, tag="msk16")
                    nc.scalar.activation(msk16[:], ps_f[2][:].rearrange("p a b -> p (a b)"),
                                         Act.Sigmoid, bias=bm_sb[:, 0:1], scale=1.0)

                    # floor (robust to convert rounding): i = int(q); f = float(i);
                    # f -= (f > q)
                    def floorb(q, tag):
                        ti = wpool.tile([72, CH], dt.int32, tag="fl_i32")
                        nc.vector.tensor_copy(ti[:], q[:])
                        tf = wpool.tile([72, CH], f32, tag="fl_f32")
                        nc.scalar.activation(tf[:], ti[:], Act.Identity)
                        gg = wpool.tile([72, CH], f32, tag="fl_gt")
                        nc.vector.tensor_tensor(gg[:], tf[:], q[:], Alu.is_gt)
                        fl = wpool.tile([72, CH], f32, tag=tag)
                        nc.vector.tensor_tensor(fl[:], tf[:], gg[:], Alu.subtract)
                        return fl

                    fyb = floorb(qyb, "fyb")
                    wy16 = wpool.tile([72, CH], f16, tag="wy16")
                    nc.vector.tensor_tensor(wy16[:], qyb[:], fyb[:], Alu.subtract)
                    fxb = floorb(qxb, "fxb")
                    wx16 = wpool.tile([72, CH], f16, tag="wx16")
                    nc.vector.tensor_tensor(wx16[:], qxb[:], fxb[:], Alu.subtract)

                    # clamp into the padded frame; out-of-frame samples land on
                    # zero rows/cols so they contribute 0 exactly like the
                    # reference's validity masking.
                    fyc = wpool.tile([72, CH], f32, tag="fyc")
                    nc.vector.tensor_scalar(fyc[:], fyb[:], FB + 128.0, FB - 2.0,
                                            Alu.min, Alu.max)
                    fxc = wpool.tile([72, CH], f32, tag="fxc")
                    nc.vector.tensor_scalar(fxc[:], fxb[:], FB + 128.0, FB - 2.0,
                                            Alu.min, Alu.max)
                    # flat padded index (fy+2)*132 + (fx+2) = base - 67830
                    base = wpool.tile([72, CH], f32, tag="base")
                    nc.vector.scalar_tensor_tensor(base[:], fyc[:], 132.0, fxc[:],
                                                   Alu.mult, Alu.add)
                    i0f = wpool.tile([72, CH], f32, tag="i0f")
                    nc.vector.tensor_scalar(i0f[:], base[:],
                                            -(FB * 132.0 + FB) + 2 * 132.0 + 2.0,
                                            None, Alu.add)

                    # i16 index streams (y0 row, y1 row), written in the wrapped
                    # gather layout: partition 16g+2*pf+yp, col k*64+s where
                    # pos = s*8+pf.  st2[p=(g,k), yp, pf, s].
                    st2v = st2.rearrange("p (yp pf s) -> p yp pf s", yp=2, pf=8)
                    cast_instrs = []
                    for yp in range(2):
                        src = i0f
                        if yp == 1:
                            i1f = wpool.tile([72, CH], f32, tag="i1f")
                            nc.vector.tensor_scalar(i1f[:], i0f[:], 132.0, None, Alu.add)
                            src = i1f
                        ti = wpool.tile([72, CH], dt.int32, tag=f"ic_i32_{yp}")
                        nc.vector.tensor_copy(ti[:], src[:])
                        d = nc.vector.tensor_copy(
                            st2v[:, yp].rearrange("p pf s -> p s pf"),
                            ti[:])
                        cast_instrs.append(d)
                    for ci in cast_instrs:
                        for pd in prev_dmas[0]:
                            add_dep_helper(ci.ins, pd.ins, True, "st2 WAR vs prev idx DMA")

                    # idx DMAs: 16 per chunk, contiguous 128B runs
                    idx_dmas = []
                    for yp in range(2):
                        for pf in range(8):
                            d = nc.sync.dma_start(idxw[2 * pf + yp::16, :],
                                                  st2v[:, yp, pf, :])
                            for ci in cast_instrs:
                                add_dep_helper(d.ins, ci.ins, True, "idx DMA RAW st2")
                            for pg in prev_gathers[0]:
                                add_dep_helper(d.ins, pg.ins, True, "idxw WAR vs prev gather")
                            idx_dmas.append(d)
                    prev_dmas[0] = idx_dmas

                    # corner weights (mask folded in), fp16, corner-major blocks
                    uy1 = wpool.tile([72, CH], f16, tag="uy1")
                    nc.vector.tensor_tensor(uy1[:], wy16[:], msk16[:], Alu.mult)
                    uy0 = wpool.tile([72, CH], f16, tag="uy0")
                    nc.vector.tensor_tensor(uy0[:], msk16[:], uy1[:], Alu.subtract)
                    ux0 = wpool.tile([72, CH], f16, tag="ux0")
                    nc.vector.tensor_scalar(ux0[:], wx16[:], -1.0, 1.0, Alu.mult, Alu.add)
                    cu = wpool.tile([72, 4, CH], f16, tag="cu")
                    nc.vector.tensor_tensor(cu[:, 0], uy0[:], ux0[:], Alu.mult)
                    nc.vector.tensor_tensor(cu[:, 1], uy0[:], wx16[:], Alu.mult)
                    nc.vector.tensor_tensor(cu[:, 2], uy1[:], ux0[:], Alu.mult)
                    nc.vector.tensor_tensor(cu[:, 3], uy1[:], wx16[:], Alu.mult)

                    # per-tap: replicate weights 72->128 via one-hot matmul,
                    # gather x-pairs, weight, accumulate 4 corner matmuls
                    gathers = []
                    dcn_ps = ppool.tile([64, CH], f32, tag="dcn_ps")
                    for kt in range(KK):
                        rep_ps = ppool.tile([128, NI * 2], f32, tag="rep_ps")
                        for q in range(4):
                            nc.tensor.matmul(
                                rep_ps[:, q * 512:(q + 1) * 512],
                                wrep_sb[:, kt * 128:(kt + 1) * 128],
                                cu[:, :, q * 128:(q + 1) * 128].rearrange(
                                    "p c pos -> p pos c"),
                                start=True, stop=True)
                        rep16 = wpool.tile([128, NI * 2], f16, tag=f"rep16_{kt % 2}")
                        nc.scalar.activation(rep16[:], rep_ps[:], Act.Copy)

                        gth = nc.gpsimd.ap_gather(
                            out_ap=g_out[:], in_ap=nbr_sb[:],
                            idxs_ap=idxw[:, kt * (NI // 16):(kt + 1) * (NI // 16)],
                            channels=128, num_elems=NE2, d=2, num_idxs=NI)
                        for d in idx_dmas:
                            add_dep_helper(gth.ins, d.ins, True, "gather RAW on idxw")
                        gathers.append(gth)

                        gw = wpool.tile([128, NI * 2], f16, tag=f"gw_{kt % 2}")
                        nc.vector.tensor_tensor(gw[:], g_out[:], rep16[:], Alu.mult)
                        gwv = gw[:].rearrange("p (pos c) -> p pos c", c=4)
                        for cidx in range(4):
                            nc.tensor.matmul(dcn_ps[:], w3_sb[:, kt * 64:(kt + 1) * 64],
                                             gwv[:, :, cidx],
                                             start=(kt == 0 and cidx == 0),
                                             stop=(kt == KK - 1 and cidx == 3))
                    prev_gathers[0] = gathers

                    oc = wpool.tile([64, CH], f32, tag="oc")
                    nc.vector.tensor_scalar(oc[:], dcn_ps[:], b3_sb[:, 0:1], None, Alu.add)
                    ob = wpool.tile([64, CH], f32, tag="ob")
                    lrelu_stt(ob[:], oc[:])
                    nc.sync.dma_start(out_d[:, c * CH:(c + 1) * CH], ob[:])

    nc.compile()
    return nc


def _prep_inputs(nbr, ref, w_off1, b_off1, w_om, b_om, w_dcn, b_dcn):
    """Build the 8 per-core input dicts.  Layout: p = g*KK + k (g-major)."""
    f16 = np.float16
    dy = np.repeat(np.arange(3) - 1, 3).astype(np.float32)  # per tap k
    dx = np.tile(np.arange(3) - 1, 3).astype(np.float32)

    # conv1 weights [128in, 64out] per tap
    w1 = np.zeros((128, KK * 64), f16)
    for kt in range(KK):
        ky, kx = kt // 3, kt % 3
        w1[:, kt * 64:(kt + 1) * 64] = w_off1[:, :, ky, kx].T.astype(f16)

    # conv2 weights, 66-channel contraction, out partition p = g*KK + k
    w2 = np.zeros((66, 3 * KK * 72), f16)
    for f in range(3):
        for kt in range(KK):
            ky, kx = kt // 3, kt % 3
            blk = np.zeros((66, 72), np.float32)
            for g in range(G):
                for k in range(KK):
                    blk[0:64, g * KK + k] = w_om[f * 72 + g * KK + k, :, ky, kx]
            if kt == 4:  # center tap carries the position maps
                if f == 0:
                    blk[64, :] = 1.0
                elif f == 1:
                    blk[65, :] = 1.0
            w2[:, (f * KK + kt) * 72:(f * KK + kt + 1) * 72] = blk.astype(f16)

    # dcn weights: partition 16g+j holds channel g*8+j (j<8), zeros for j>=8
    w3 = np.zeros((128, KK * 64), f16)
    wd = w_dcn.reshape(64, G, CG, 3, 3)
    for kt in range(KK):
        ky, kx = kt // 3, kt % 3
        blk = np.zeros((128, 64), np.float32)
        for g in range(G):
            for j in range(CG):
                blk[16 * g + j, :] = wd[:, g, j, ky, kx]
        w3[:, kt * 64:(kt + 1) * 64] = blk.astype(f16)

    # one-hot replication p=(g,k) -> m=16g+j for tap k
    wrep = np.zeros((72, KK * 128), f16)
    for kt in range(KK):
        for m in range(128):
            g = m // 16
            wrep[g * KK + kt, kt * 128 + m] = 1.0

    by = np.zeros((72, 1), np.float32)
    bx = np.zeros((72, 1), np.float32)
    bm = np.zeros((72, 1), np.float32)
    for g in range(G):
        for k in range(KK):
            p = g * KK + k
            by[p, 0] = b_om[0 * 72 + g * KK + k] + dy[k] + FB
            bx[p, 0] = b_om[1 * 72 + g * KK + k] + dx[k] + FB
            bm[p, 0] = b_om[2 * 72 + g * KK + k]
    b1 = b_off1.reshape(64, 1).astype(np.float32)
    b3 = b_dcn.reshape(64, 1).astype(np.float32)

    in_maps = []
    for core in range(N_CORES):
        b = core // 2
        s = (core % 2) * HALF
        # conv input: concat channels, rows [s-2, s+66), zero pad, 130 cols
        ci = np.zeros((128, 68, 130), f16)
        cat = np.concatenate([nbr[b], ref[b]], axis=0)  # [128, H, W]
        r_lo, r_hi = s - 2, s + 66
        src_lo, src_hi = max(r_lo, 0), min(r_hi, H)
        ci[:, src_lo - r_lo: src_hi - r_lo, 1:129] = cat[:, src_lo:src_hi, :].astype(f16)

        # padded gather source with x-pair duplication
        pimg = np.zeros((128, PW, PW), np.float32)
        for g in range(G):
            for j in range(16):
                pimg[16 * g + j, PAD:PAD + H, PAD:PAD + W] = nbr[b, CG * g + (j % CG)]
        flat = pimg.reshape(128, NE2)
        ng2 = np.zeros((128, NE2, 2), f16)
        ng2[:, :, 0] = flat.astype(f16)
        ng2[:, :-1, 1] = flat[:, 1:].astype(f16)

        # row/col maps for conv2's center tap: rmap[j,w] = s-1+j, cmap = w-1
        rw = np.zeros((2, 66, 130), f16)
        rw[0] = (s - 1 + np.arange(66, dtype=np.float32))[:, None]
        rw[1] = (np.arange(130, dtype=np.float32) - 1)[None, :]

        e0 = np.full((64, 1), 0.0 if s == 0 else 1.0, np.float32)
        e65 = np.full((64, 1), 0.0 if s + HALF == H else 1.0, np.float32)
        in_maps.append(dict(
            conv_in=ci.reshape(128, -1), nbr_g=ng2.reshape(128, -1),
            w1=w1, w2=w2, w3=w3, wrep=wrep, rwmap=rw.reshape(2, -1),
            by=by, bx=bx, bm=bm, b1=b1, b3=b3, e0=e0, e65=e65,
        ))
    return in_maps


def kernel(**inputs):
    global _compiled
    from concourse.bass_utils import run_bass_kernel_spmd

    if _compiled is None:
        _compiled = _build_program()
    nc = _compiled

    in_maps = _prep_inputs(
        inputs["nbr_fea_l"], inputs["ref_fea_l"], inputs["w_off1"],
        inputs["b_off1"], inputs["w_om"], inputs["b_om"],
        inputs["w_dcn"], inputs["b_dcn"])

    res = run_bass_kernel_spmd(nc, in_maps, core_ids=list(range(N_CORES)))
    out = np.zeros((B, NF, H, W), np.float32)
    for core in range(N_CORES):
        b = core // 2
        s = (core % 2) * HALF
        out[b, :, s:s + HALF, :] = res.results[core]["out"].reshape(64, HALF, W)
    return out


if __name__ == "__main__":
    print("smoke build only")
    _build_program()
    print("build ok")
